# revision 15
# baseline (speedup 1.0000x reference)
"""DeepSeek-style MLA decode attention (batch=8, 128 heads, cache 512) on 8 NeuronCores.

Sharding: tensor-parallel over heads (16 heads/core), all streamed tensors
host-cast to bf16 (halves HBM traffic; the kernel is memory-bound).

 - W_down ([Wq_down | Wkv_down], 7168x2048) row-sharded: each core computes a
   partial c = x_rows @ W_down_rows and a tiny (8,2048) f32 AllReduce completes
   it (replaces the baseline's big q ReduceScatter that serialized everything).
 - Wq_up / Wv_up column-sharded by head; q/v_new computed fully on the owner.
 - Scores accumulate into a single PSUM bank with no per-product DVE work:
   Zbig [128, 16384] is all-zero except columns 127 + 128*hb which hold q_hb
   (built with ONE strided DVE copy). The lhsT window
   Zbig[:, 127*hb+127 : 127*hb+255] then contains exactly one live column at
   position hb, so matmul hb writes score row hb and exact zeros elsewhere.
 - Wo packed chunk-major [28, 128, 8*512]: each 512-col output chunk
   accumulates over head-blocks in one bank, chunks finish progressively, and
   the output ReduceScatters in two pieces (10+4 chunks) so the first overlaps
   compute and the last is a tiny 64KB op right at stream end.
 - One elastic 16-slot SBUF pool streams wd/wq/wv/k/v/wo tiles in consumption
   order, so the ~30-70us core-start skew absorbed by the first collective is
   ridden out with ~16MB of useful prefetch.

The reference's "new token" softmax is over a length-1 axis (== 1.0), so
k_new/Wk_up are dead and the new-token contribution is simply + v_new.
"""

import ml_dtypes
import numpy as np

import concourse.mybir as mybir
import concourse.tile as tile
from concourse import bacc
from concourse import bass_utils
from concourse.masks import make_identity

NC_ = 8                      # cores
B = 8                        # batch
H = 128                      # total heads
HP = H // NC_                # 16 heads per core
D = 128                      # head dim
L = 512                      # cache len
HID = 7168
QL = 1536
KVL = 512
NH = HP * D                  # 2048 per-core head cols
HROWS = HID // NC_           # 896 hidden rows per core for W_down
SCALE = 1.0 / float(np.sqrt(D))
F32 = mybir.dt.float32
BF16 = mybir.dt.bfloat16
NPBF16 = ml_dtypes.bfloat16


def build_nc():
    nc = bacc.Bacc(
        "TRN2",
        target_bir_lowering=False,
        debug=False,
        enable_asserts=True,
        num_devices=NC_,
    )
    xt = nc.dram_tensor("xt", [128, 7 * B], BF16, kind="ExternalInput").ap()
    wd = nc.dram_tensor("wd", [128, 7 * 2048], BF16, kind="ExternalInput").ap()
    wq = nc.dram_tensor("wq", [128, 12 * 2048], BF16, kind="ExternalInput").ap()
    wv = nc.dram_tensor("wv", [128, 4 * 2048], BF16, kind="ExternalInput").ap()
    kt = nc.dram_tensor("kt", [16, 128, 4096], BF16, kind="ExternalInput").ap()
    v = nc.dram_tensor("v", [16, 128, 4096], BF16, kind="ExternalInput").ap()
    wo = nc.dram_tensor("wo", [28, 128, 4096], BF16, kind="ExternalInput").ap()
    o = nc.dram_tensor("o", [1, HID], F32, kind="ExternalOutput").ap()

    rg = [list(range(NC_))]

    with tile.TileContext(nc) as tc:
        with (
            tc.tile_pool(name="const", bufs=1) as constp,
            tc.tile_pool(name="sbuf", bufs=1) as sb,
            tc.tile_pool(name="streamp", bufs=16) as streamp,
            tc.tile_pool(name="psbank", bufs=6, space="PSUM") as psbank,
            tc.tile_pool(name="pstr", bufs=2, space="PSUM") as pstr,
            tc.tile_pool(name="dram", bufs=1, space="DRAM") as dram,
        ):
            ident = constp.tile([128, 128], F32)
            make_identity(nc, ident[:])
            id8 = ident[0:8, 0:8]
            # uint8 one-hot columns for CopyPredicated masks (must be int dtype)
            identu8 = constp.tile([128, 128], mybir.dt.uint8, tag="identu8")
            nc.vector.tensor_copy(identu8[:], ident[:])
            # sparse masked-lhsT buffer for phase A: live cols at 127 + 128*hb
            zbig = constp.tile([128, 16384], BF16, tag="zbig")
            nc.vector.memset(zbig[:], 0)

            # ---------------- partial cdown = x_rows @ [Wq_down | Wkv_down]_rows ----
            xt_sb = constp.tile([128, 7 * B], BF16, tag="xt")
            nc.sync.dma_start(out=xt_sb[:], in_=xt)
            ps_cd = [
                psbank.tile([8, 512], F32, tag="bank", name=f"ps_cd{i}")
                for i in range(4)
            ]
            for i in range(7):
                wd_t = streamp.tile([128, 2048], BF16, tag="stream", name="wd_t")
                nc.sync.dma_start(out=wd_t[:], in_=wd[:, i * 2048:(i + 1) * 2048])
                lhs = xt_sb[:, i * B:(i + 1) * B]
                for j in range(4):
                    nc.tensor.matmul(
                        ps_cd[j][:8, :],
                        lhs,
                        wd_t[:, j * 512:(j + 1) * 512],
                        start=(i == 0), stop=(i == 6),
                    )
            cdp_sb = sb.tile([8, 2048], F32, tag="low8", name="cdp_sb")
            for j in range(4):
                nc.vector.tensor_copy(cdp_sb[:, j * 512:(j + 1) * 512], ps_cd[j][:8, :])

            # tiny AllReduce completes cdown on every core (~64KB, mesh algo)
            cd_p = dram.tile([8, 2048], F32, tag="cdp_d")
            nc.gpsimd.dma_start(out=cd_p[:], in_=cdp_sb[:])
            cd_r = dram.tile([8, 2048], F32, tag="cdr_d")
            nc.gpsimd.collective_compute(
                "AllReduce",
                mybir.AluOpType.add,
                replica_groups=rg,
                ins=[cd_p.opt()],
                outs=[cd_r.opt()],
            )
            cdown = sb.tile([8, 2048], F32, tag="low8", name="cdown")
            nc.gpsimd.dma_start(out=cdown[:], in_=cd_r[:])

            # transpose cdown -> cdT [128, 128] bf16: cols j*8 hold chunk j of
            # cq (j<12) / ckv (j>=12)
            ps_cdT = pstr.tile([128, 128], F32, tag="tr")
            for j in range(16):
                nc.tensor.transpose(
                    ps_cdT[0:128, j * 8:(j + 1) * 8],
                    cdown[:, j * 128:(j + 1) * 128],
                    id8,
                )
            cdT = sb.tile([128, 128], BF16, tag="cdT")
            nc.vector.tensor_copy(cdT[:], ps_cdT[:])

            # ---------------- q = cq @ Wq_up_c  (8, 2048) ----------------
            ps_q = [
                psbank.tile([8, 512], F32, tag="bank", name=f"ps_q{i}")
                for i in range(4)
            ]
            for jj in range(6):
                wq_t = streamp.tile([128, 4096], BF16, tag="stream", name="wq_t")
                nc.sync.dma_start(out=wq_t[:], in_=wq[:, jj * 4096:(jj + 1) * 4096])
                for s in range(2):
                    j = jj * 2 + s
                    lhsT = cdT[:, j * 8:(j + 1) * 8]
                    for i in range(4):
                        nc.tensor.matmul(
                            ps_q[i][:8, :],
                            lhsT,
                            wq_t[:, s * 2048 + i * 512:s * 2048 + (i + 1) * 512],
                            start=(j == 0), stop=(j == 11),
                        )
            qown = sb.tile([8, 2048], F32, tag="low8", name="qown")
            for i in range(4):
                nc.vector.tensor_copy(qown[:, i * 512:(i + 1) * 512], ps_q[i][:8, :])

            # qT [128 d, hb] via 16 transposes (hb = h*8 + b)
            ps_qT = pstr.tile([128, 128], F32, tag="tr")
            for h in range(HP):
                nc.tensor.transpose(
                    ps_qT[0:128, h * 8:(h + 1) * 8],
                    qown[:, h * D:(h + 1) * D],
                    id8,
                )
            qT = sb.tile([128, 128], BF16, tag="qT")
            nc.vector.tensor_copy(qT[:], ps_qT[:])
            # scatter qT columns into zbig live columns (one strided copy)
            zview = zbig[:].rearrange("p (n s) -> p n s", s=128)
            nc.vector.tensor_copy(
                zview[:, :, 127:128],
                qT[:].rearrange("p (n o) -> p n o", o=1),
            )

            # ---------------- v_new = ckv @ Wv_up_c  (8, 2048) ----------------
            # wv streams through the wq pool (2 chunks of 2 rank-blocks each)
            ps_v = [
                psbank.tile([8, 512], F32, tag="bank", name=f"ps_v{i}")
                for i in range(4)
            ]
            for jj in range(2):
                wv_t = streamp.tile([128, 4096], BF16, tag="stream", name=f"wv{jj}")
                nc.sync.dma_start(out=wv_t[:], in_=wv[:, jj * 4096:(jj + 1) * 4096])
                for s in range(2):
                    j = jj * 2 + s
                    lhsT = cdT[:, 96 + j * 8:96 + (j + 1) * 8]
                    for i in range(4):
                        nc.tensor.matmul(
                            ps_v[i][:8, :],
                            lhsT,
                            wv_t[:, s * 2048 + i * 512:s * 2048 + (i + 1) * 512],
                            start=(j == 0), stop=(j == 3),
                        )
            vnew = sb.tile([8, 2048], F32, tag="low8", name="vnew")
            for i in range(4):
                nc.vector.tensor_copy(vnew[:, i * 512:(i + 1) * 512], ps_v[i][:8, :])
            ps_vT = pstr.tile([128, 128], F32, tag="tr")
            for h in range(HP):
                nc.tensor.transpose(
                    ps_vT[0:128, h * 8:(h + 1) * 8],
                    vnew[:, h * D:(h + 1) * D],
                    id8,
                )
            vnewT = sb.tile([128, 128], F32, tag="vnewT")
            nc.vector.tensor_copy(vnewT[:], ps_vT[:])

            # ---------------- phase A: scores over k cache ----------------
            # kt tile t = head t's keys for all batches: [128 d, (b, keys)].
            # All 128 products accumulate into ONE bank; product hb's lhsT
            # window holds q_hb at column hb and zeros elsewhere, so it writes
            # row hb and adds exact zeros to every other row. Pure PE phase.
            ps_sc = psbank.tile([128, 512], F32, tag="bank", name="score_bank")
            for t in range(HP):
                kt_t = streamp.tile([128, 4096], BF16, tag="stream", name="kt_t")
                nc.sync.dma_start(out=kt_t[:], in_=kt[t])
                for u in range(8):
                    hb = 8 * t + u
                    nc.tensor.matmul(
                        ps_sc[:],
                        zbig[:, 127 * hb + 127:127 * hb + 255],
                        kt_t[:, u * 512:(u + 1) * 512],
                        start=(hb == 0), stop=(hb == 127),
                    )

            # softmax: unnormalized probs = exp(scale * scores) straight off
            # the bank; fold 1/denom into the attn rows after phase B.
            probs = sb.tile([128, 512], F32, tag="probs")
            denom = sb.tile([128, 1], F32, tag="denom")
            nc.scalar.activation(
                probs[:], ps_sc[:], mybir.ActivationFunctionType.Exp,
                scale=SCALE, accum_out=denom[:],
            )
            recip = sb.tile([128, 1], F32, tag="recip")
            nc.vector.reciprocal(recip[:], denom[:])

            ps_pT = psbank.tile([128, 512], F32, tag="bank")
            for cc in range(4):
                nc.tensor.transpose(
                    ps_pT[:, cc * 128:(cc + 1) * 128],
                    probs[:, cc * 128:(cc + 1) * 128],
                    ident[:],
                )
            probsT = sb.tile([128, 512], BF16, tag="probsT")
            nc.vector.tensor_copy(probsT[:], ps_pT[:])

            # ---------------- phase B: attn rows = probs @ V ----------------
            # v tile t: [128 l-in-chunk, (chunk cc, b, d)]; accumulate over cc,
            # extract row 8t+uu*4+w from column block w.
            attn = sb.tile([128, 128], F32, tag="attn")
            for t in range(HP):
                v_t = streamp.tile([128, 4096], BF16, tag="stream", name="v_t")
                nc.sync.dma_start(out=v_t[:], in_=v[t])
                for uu in range(2):
                    ps_a = psbank.tile([128, 512], F32, tag="bank")
                    for cc in range(4):
                        nc.tensor.matmul(
                            ps_a[:],
                            probsT[:, cc * 128:(cc + 1) * 128],
                            v_t[:, cc * 1024 + uu * 512:cc * 1024 + (uu + 1) * 512],
                            start=(cc == 0), stop=(cc == 3),
                        )
                    for w in range(4):
                        hb = 8 * t + uu * 4 + w
                        nc.vector.copy_predicated(
                            attn[:],
                            identu8[:, hb:hb + 1].broadcast_to((128, 128)),
                            ps_a[:, w * 128:(w + 1) * 128],
                        )

            # normalize rows, transpose, add v_new^T, cast bf16
            attn_n = sb.tile([128, 128], F32, tag="attn_n")
            nc.vector.tensor_scalar_mul(attn_n[:], attn[:], recip[:])
            ps_aT = pstr.tile([128, 128], F32, tag="tr")
            nc.tensor.transpose(ps_aT[:], attn_n[:], ident[:])
            attnT = sb.tile([128, 128], BF16, tag="attnT")
            nc.vector.tensor_add(attnT[:], ps_aT[:], vnewT[:])

            # ---------------- phase C: o_part = attn^T @ Wo_c ----------------
            # Chunk-major: per (half n, chunk i) one 2MB DMA [128, 16*512] and
            # one accumulating bank over the 16 head-blocks. Chunks complete
            # progressively; each half stores once (HWDGE, so it is not queued
            # behind the previous collective's wait) and ReduceScatters in bf16
            # over the batch dim (core b keeps batch b's row).
            ci0 = 0
            for n, nch in ((0, 6), (1, 8)):
                obuf = sb.tile([8, nch * 512], F32, tag="obuf", name=f"obuf{n}")
                for i in range(nch):
                    ci = ci0 + i
                    ps_o = psbank.tile([8, 512], F32, tag="bank")
                    for hh in range(2):
                        wo_t = streamp.tile([128, 4096], BF16, tag="stream", name="wo_t")
                        nc.sync.dma_start(out=wo_t[:], in_=wo[ci * 2 + hh])
                        for h8 in range(8):
                            h = hh * 8 + h8
                            nc.tensor.matmul(
                                ps_o[:8, :],
                                attnT[:, h * 8:(h + 1) * 8],
                                wo_t[:, h8 * 512:(h8 + 1) * 512],
                                start=(h == 0), stop=(h == HP - 1),
                            )
                    nc.vector.tensor_copy(obuf[:, i * 512:(i + 1) * 512], ps_o[:8, :])
                ob = dram.tile([B, nch * 512], F32, tag=f"ob{n}", name=f"ob{n}")
                nc.sync.dma_start(out=ob[:], in_=obuf[:])
                ors = dram.tile([1, nch * 512], F32, tag=f"ors{n}", name=f"ors{n}")
                nc.gpsimd.collective_compute(
                    "ReduceScatter",
                    mybir.AluOpType.add,
                    replica_groups=rg,
                    ins=[ob.opt()],
                    outs=[ors.opt()],
                )
                nc.gpsimd.dma_start(
                    out=o[:, ci0 * 512:(ci0 + nch) * 512], in_=ors[:]
                )
                ci0 += nch

    nc.compile()
    return nc


_NC_CACHE = None


def _get_nc():
    global _NC_CACHE
    if _NC_CACHE is None:
        _NC_CACHE = build_nc()
    return _NC_CACHE


def make_in_maps(x, k_cache, v_cache, Wq_down, Wq_up, Wkv_down, Wv_up, Wo):
    x2 = np.asarray(x, dtype=np.float32).reshape(B, HID)
    k_cache = np.asarray(k_cache, dtype=np.float32)
    v_cache = np.asarray(v_cache, dtype=np.float32)
    wd_full = np.concatenate(
        [np.asarray(Wq_down, dtype=np.float32), np.asarray(Wkv_down, dtype=np.float32)],
        axis=1,
    )  # [7168, 2048]
    Wq_up = np.asarray(Wq_up, dtype=np.float32)
    Wv_up = np.asarray(Wv_up, dtype=np.float32)
    Wo = np.asarray(Wo, dtype=np.float32)

    in_maps = []
    for c in range(NC_):
        hs = slice(c * HP, (c + 1) * HP)
        cols = slice(c * NH, (c + 1) * NH)
        rows = slice(c * HROWS, (c + 1) * HROWS)
        xt_c = np.ascontiguousarray(
            x2[:, rows].T.reshape(7, 128, B).transpose(1, 0, 2).reshape(128, 7 * B)
        ).astype(NPBF16)
        wd_c = np.ascontiguousarray(
            wd_full[rows].reshape(7, 128, 2048).transpose(1, 0, 2).reshape(128, 7 * 2048)
        ).astype(NPBF16)
        wq_c = np.ascontiguousarray(
            Wq_up[:, cols].reshape(12, 128, 2048).transpose(1, 0, 2).reshape(128, 12 * 2048)
        ).astype(NPBF16)
        wv_c = np.ascontiguousarray(
            Wv_up[:, cols].reshape(4, 128, 2048).transpose(1, 0, 2).reshape(128, 4 * 2048)
        ).astype(NPBF16)
        # kt tile t = head t: [d, (b, keys)]
        kt_c = np.ascontiguousarray(
            k_cache[:, hs].transpose(1, 3, 0, 2).reshape(16, 128, 4096)
        ).astype(NPBF16)
        # v tile t = head t: [l-in-chunk, (chunk, b, d)]
        v_c = np.ascontiguousarray(
            v_cache[:, hs]                     # (8 b, 16 h, 512 l, 128 d)
            .reshape(B, HP, 4, 128, 128)       # [b, h, cc, l, d]
            .transpose(1, 3, 2, 0, 4)          # [h, l, cc, b, d]
            .reshape(16, 128, 4096)
        ).astype(NPBF16)
        # wo chunk-major halves: [(n,i,hh) 28, d 128, (h8, 512)]
        wo_c = np.ascontiguousarray(
            Wo[cols].reshape(2, 8, 128, 14, 512)   # [hh, h8, d, k, j]
            .transpose(3, 0, 2, 1, 4)              # [k, hh, d, h8, j]
            .reshape(28, 128, 4096)
        ).astype(NPBF16)
        in_maps.append(
            {
                "xt": xt_c,
                "wd": wd_c,
                "wq": wq_c,
                "wv": wv_c,
                "kt": kt_c,
                "v": v_c,
                "wo": wo_c,
            }
        )
    return in_maps


def kernel(x, k_cache, v_cache, Wq_down, Wq_up, Wkv_down, Wk_up, Wv_up, Wo, **_):
    in_maps = make_in_maps(
        x, k_cache, v_cache, Wq_down, Wq_up, Wkv_down, Wv_up, Wo
    )
    nc = _get_nc()
    res = bass_utils.run_bass_kernel_spmd(nc, in_maps, core_ids=list(range(NC_)))
    out = np.stack([res.results[b]["o"] for b in range(B)], axis=0)  # (8, 1, 7168)
    return np.ascontiguousarray(out, dtype=np.float32)


# revision 16
# speedup vs baseline: 1.0777x; 1.0777x over previous
"""DeepSeek-style MLA decode attention (batch=8, 128 heads, cache 512) on 8 NeuronCores.

Sharding: tensor-parallel over heads (16 heads/core), all streamed tensors
host-cast to bf16 (halves HBM traffic; the kernel is memory-bound).

 - W_down ([Wq_down | Wkv_down], 7168x2048) row-sharded: each core computes a
   partial c = x_rows @ W_down_rows and a tiny (8,2048) f32 AllReduce completes
   it (replaces the baseline's big q ReduceScatter that serialized everything).
 - Wq_up / Wv_up column-sharded by head; q/v_new computed fully on the owner.
 - Scores accumulate into a single PSUM bank with no per-product DVE work:
   Zbig [128, 16384] is all-zero except columns 127 + 128*hb which hold q_hb
   (built with ONE strided DVE copy). The lhsT window
   Zbig[:, 127*hb+127 : 127*hb+255] then contains exactly one live column at
   position hb, so matmul hb writes score row hb and exact zeros elsewhere.
 - Wo packed chunk-major [28, 128, 8*512]: each 512-col output chunk
   accumulates over head-blocks in one bank, chunks finish progressively, and
   the output ReduceScatters in two pieces (10+4 chunks) so the first overlaps
   compute and the last is a tiny 64KB op right at stream end.
 - One elastic 16-slot SBUF pool streams wd/wq/wv/k/v/wo tiles in consumption
   order, so the ~30-70us core-start skew absorbed by the first collective is
   ridden out with ~16MB of useful prefetch.

The reference's "new token" softmax is over a length-1 axis (== 1.0), so
k_new/Wk_up are dead and the new-token contribution is simply + v_new.
"""

import ml_dtypes
import numpy as np

import concourse.mybir as mybir
import concourse.tile as tile
from concourse import bacc
from concourse import bass_utils
from concourse.masks import make_identity

NC_ = 8                      # cores
B = 8                        # batch
H = 128                      # total heads
HP = H // NC_                # 16 heads per core
D = 128                      # head dim
L = 512                      # cache len
HID = 7168
QL = 1536
KVL = 512
NH = HP * D                  # 2048 per-core head cols
HROWS = HID // NC_           # 896 hidden rows per core for W_down
SCALE = 1.0 / float(np.sqrt(D))
F32 = mybir.dt.float32
BF16 = mybir.dt.bfloat16
NPBF16 = ml_dtypes.bfloat16


def build_nc():
    nc = bacc.Bacc(
        "TRN2",
        target_bir_lowering=False,
        debug=False,
        enable_asserts=True,
        num_devices=NC_,
    )
    xt = nc.dram_tensor("xt", [128, 7 * B], BF16, kind="ExternalInput").ap()
    wd = nc.dram_tensor("wd", [128, 7 * 2048], BF16, kind="ExternalInput").ap()
    wq = nc.dram_tensor("wq", [128, 12 * 2048], BF16, kind="ExternalInput").ap()
    wv = nc.dram_tensor("wv", [128, 4 * 2048], BF16, kind="ExternalInput").ap()
    kt = nc.dram_tensor("kt", [16, 128, 4096], BF16, kind="ExternalInput").ap()
    v = nc.dram_tensor("v", [16, 128, 4096], BF16, kind="ExternalInput").ap()
    wo = nc.dram_tensor("wo", [28, 128, 4096], BF16, kind="ExternalInput").ap()
    o = nc.dram_tensor("o", [1, HID], F32, kind="ExternalOutput").ap()

    rg = [list(range(NC_))]

    with tile.TileContext(nc) as tc:
        with (
            tc.tile_pool(name="const", bufs=1) as constp,
            tc.tile_pool(name="sbuf", bufs=1) as sb,
            tc.tile_pool(name="streamp", bufs=16) as streamp,
            tc.tile_pool(name="psbank", bufs=6, space="PSUM") as psbank,
            tc.tile_pool(name="pstr", bufs=2, space="PSUM") as pstr,
            tc.tile_pool(name="dram", bufs=1, space="DRAM") as dram,
        ):
            ident = constp.tile([128, 128], F32)
            make_identity(nc, ident[:])
            id8 = ident[0:8, 0:8]
            # uint8 one-hot columns for CopyPredicated masks (must be int dtype)
            identu8 = constp.tile([128, 128], mybir.dt.uint8, tag="identu8")
            nc.vector.tensor_copy(identu8[:], ident[:])
            # sparse masked-lhsT buffer for phase A: live cols at 127 + 128*hb
            zbig = constp.tile([128, 16384], BF16, tag="zbig")
            nc.vector.memset(zbig[:], 0)

            # ---------------- partial cdown = x_rows @ [Wq_down | Wkv_down]_rows ----
            xt_sb = constp.tile([128, 7 * B], BF16, tag="xt")
            nc.sync.dma_start(out=xt_sb[:], in_=xt)
            ps_cd = [
                psbank.tile([8, 512], F32, tag="bank", name=f"ps_cd{i}")
                for i in range(4)
            ]
            for i in range(7):
                wd_t = streamp.tile([128, 2048], BF16, tag="stream", name="wd_t")
                nc.sync.dma_start(out=wd_t[:], in_=wd[:, i * 2048:(i + 1) * 2048])
                lhs = xt_sb[:, i * B:(i + 1) * B]
                for j in range(4):
                    nc.tensor.matmul(
                        ps_cd[j][:8, :],
                        lhs,
                        wd_t[:, j * 512:(j + 1) * 512],
                        start=(i == 0), stop=(i == 6),
                    )
            cdp_sb = sb.tile([8, 2048], F32, tag="low8", name="cdp_sb")
            for j in range(4):
                nc.vector.tensor_copy(cdp_sb[:, j * 512:(j + 1) * 512], ps_cd[j][:8, :])

            # tiny AllReduce completes cdown on every core (~64KB, mesh algo)
            cd_p = dram.tile([8, 2048], F32, tag="cdp_d")
            nc.gpsimd.dma_start(out=cd_p[:], in_=cdp_sb[:])
            cd_r = dram.tile([8, 2048], F32, tag="cdr_d")
            nc.gpsimd.collective_compute(
                "AllReduce",
                mybir.AluOpType.add,
                replica_groups=rg,
                ins=[cd_p.opt()],
                outs=[cd_r.opt()],
            )
            cdown = sb.tile([8, 2048], F32, tag="low8", name="cdown")
            nc.gpsimd.dma_start(out=cdown[:], in_=cd_r[:])

            # transpose cdown -> cdT [128, 128] bf16: cols j*8 hold chunk j of
            # cq (j<12) / ckv (j>=12)
            ps_cdT = pstr.tile([128, 128], F32, tag="tr")
            for j in range(16):
                nc.tensor.transpose(
                    ps_cdT[0:128, j * 8:(j + 1) * 8],
                    cdown[:, j * 128:(j + 1) * 128],
                    id8,
                )
            cdT = sb.tile([128, 128], BF16, tag="cdT")
            nc.vector.tensor_copy(cdT[:], ps_cdT[:])

            # ---------------- q = cq @ Wq_up_c  (8, 2048) ----------------
            ps_q = [
                psbank.tile([8, 512], F32, tag="bank", name=f"ps_q{i}")
                for i in range(4)
            ]
            for jj in range(6):
                wq_t = streamp.tile([128, 4096], BF16, tag="stream", name="wq_t")
                nc.sync.dma_start(out=wq_t[:], in_=wq[:, jj * 4096:(jj + 1) * 4096])
                for s in range(2):
                    j = jj * 2 + s
                    lhsT = cdT[:, j * 8:(j + 1) * 8]
                    for i in range(4):
                        nc.tensor.matmul(
                            ps_q[i][:8, :],
                            lhsT,
                            wq_t[:, s * 2048 + i * 512:s * 2048 + (i + 1) * 512],
                            start=(j == 0), stop=(j == 11),
                        )
            qown = sb.tile([8, 2048], F32, tag="low8", name="qown")
            for i in range(4):
                nc.vector.tensor_copy(qown[:, i * 512:(i + 1) * 512], ps_q[i][:8, :])

            # qT [128 d, hb] via 16 transposes (hb = h*8 + b)
            ps_qT = pstr.tile([128, 128], F32, tag="tr")
            for h in range(HP):
                nc.tensor.transpose(
                    ps_qT[0:128, h * 8:(h + 1) * 8],
                    qown[:, h * D:(h + 1) * D],
                    id8,
                )
            qT = sb.tile([128, 128], BF16, tag="qT")
            nc.vector.tensor_copy(qT[:], ps_qT[:])
            # scatter qT columns into zbig live columns (one strided copy)
            zview = zbig[:].rearrange("p (n s) -> p n s", s=128)
            nc.vector.tensor_copy(
                zview[:, :, 127:128],
                qT[:].rearrange("p (n o) -> p n o", o=1),
            )

            # ---------------- v_new = ckv @ Wv_up_c  (8, 2048) ----------------
            # wv streams through the wq pool (2 chunks of 2 rank-blocks each)
            ps_v = [
                psbank.tile([8, 512], F32, tag="bank", name=f"ps_v{i}")
                for i in range(4)
            ]
            for jj in range(2):
                wv_t = streamp.tile([128, 4096], BF16, tag="stream", name=f"wv{jj}")
                nc.sync.dma_start(out=wv_t[:], in_=wv[:, jj * 4096:(jj + 1) * 4096])
                for s in range(2):
                    j = jj * 2 + s
                    lhsT = cdT[:, 96 + j * 8:96 + (j + 1) * 8]
                    for i in range(4):
                        nc.tensor.matmul(
                            ps_v[i][:8, :],
                            lhsT,
                            wv_t[:, s * 2048 + i * 512:s * 2048 + (i + 1) * 512],
                            start=(j == 0), stop=(j == 3),
                        )
            vnew = sb.tile([8, 2048], F32, tag="low8", name="vnew")
            for i in range(4):
                nc.vector.tensor_copy(vnew[:, i * 512:(i + 1) * 512], ps_v[i][:8, :])
            ps_vT = pstr.tile([128, 128], F32, tag="tr")
            for h in range(HP):
                nc.tensor.transpose(
                    ps_vT[0:128, h * 8:(h + 1) * 8],
                    vnew[:, h * D:(h + 1) * D],
                    id8,
                )
            vnewT = sb.tile([128, 128], F32, tag="vnewT")
            nc.vector.tensor_copy(vnewT[:], ps_vT[:])

            # ---------------- phase A: scores over k cache ----------------
            # kt tile t = head t's keys for all batches: [128 d, (b, keys)].
            # All 128 products accumulate into ONE bank; product hb's lhsT
            # window holds q_hb at column hb and zeros elsewhere, so it writes
            # row hb and adds exact zeros to every other row. Pure PE phase.
            ps_sc = psbank.tile([128, 512], F32, tag="bank", name="score_bank")
            for t in range(HP):
                kt_t = streamp.tile([128, 4096], BF16, tag="stream", name="kt_t")
                nc.sync.dma_start(out=kt_t[:], in_=kt[t])
                for u in range(8):
                    hb = 8 * t + u
                    nc.tensor.matmul(
                        ps_sc[:],
                        zbig[:, 127 * hb + 127:127 * hb + 255],
                        kt_t[:, u * 512:(u + 1) * 512],
                        start=(hb == 0), stop=(hb == 127),
                    )

            # softmax: unnormalized probs = exp(scale * scores) straight off
            # the bank; fold 1/denom into the attn rows after phase B.
            probs = sb.tile([128, 512], F32, tag="probs")
            denom = sb.tile([128, 1], F32, tag="denom")
            nc.scalar.activation(
                probs[:], ps_sc[:], mybir.ActivationFunctionType.Exp,
                scale=SCALE, accum_out=denom[:],
            )
            recip = sb.tile([128, 1], F32, tag="recip")
            nc.vector.reciprocal(recip[:], denom[:])

            ps_pT = psbank.tile([128, 512], F32, tag="bank")
            for cc in range(4):
                nc.tensor.transpose(
                    ps_pT[:, cc * 128:(cc + 1) * 128],
                    probs[:, cc * 128:(cc + 1) * 128],
                    ident[:],
                )
            probsT = sb.tile([128, 512], BF16, tag="probsT")
            nc.vector.tensor_copy(probsT[:], ps_pT[:])

            # ---------------- phase B: attn rows = probs @ V ----------------
            # v tile t: [128 l-in-chunk, (chunk cc, b, d)]; accumulate over cc,
            # extract row 8t+uu*4+w from column block w.
            attn = sb.tile([128, 128], F32, tag="attn")
            for t in range(HP):
                v_t = streamp.tile([128, 4096], BF16, tag="stream", name="v_t")
                nc.sync.dma_start(out=v_t[:], in_=v[t])
                for uu in range(2):
                    ps_a = psbank.tile([128, 512], F32, tag="bank")
                    for cc in range(4):
                        nc.tensor.matmul(
                            ps_a[:],
                            probsT[:, cc * 128:(cc + 1) * 128],
                            v_t[:, cc * 1024 + uu * 512:cc * 1024 + (uu + 1) * 512],
                            start=(cc == 0), stop=(cc == 3),
                        )
                    for w in range(4):
                        hb = 8 * t + uu * 4 + w
                        nc.vector.copy_predicated(
                            attn[:],
                            identu8[:, hb:hb + 1].broadcast_to((128, 128)),
                            ps_a[:, w * 128:(w + 1) * 128],
                        )

            # normalize rows, transpose, add v_new^T, cast bf16
            attn_n = sb.tile([128, 128], F32, tag="attn_n")
            nc.vector.tensor_scalar_mul(attn_n[:], attn[:], recip[:])
            ps_aT = pstr.tile([128, 128], F32, tag="tr")
            nc.tensor.transpose(ps_aT[:], attn_n[:], ident[:])
            attnT = sb.tile([128, 128], BF16, tag="attnT")
            nc.vector.tensor_add(attnT[:], ps_aT[:], vnewT[:])

            # ---------------- phase C: o_part = attn^T @ Wo_c ----------------
            # Chunk-major: per (half n, chunk i) one 2MB DMA [128, 16*512] and
            # one accumulating bank over the 16 head-blocks. Chunks complete
            # progressively; each half stores once (HWDGE, so it is not queued
            # behind the previous collective's wait) and ReduceScatters in bf16
            # over the batch dim (core b keeps batch b's row).
            ci0 = 0
            for n, nch in ((0, 10), (1, 4)):
                obuf = sb.tile([8, nch * 512], F32, tag="obuf", name=f"obuf{n}")
                for i in range(nch):
                    ci = ci0 + i
                    ps_o = psbank.tile([8, 512], F32, tag="bank")
                    for hh in range(2):
                        wo_t = streamp.tile([128, 4096], BF16, tag="stream", name="wo_t")
                        nc.sync.dma_start(out=wo_t[:], in_=wo[ci * 2 + hh])
                        for h8 in range(8):
                            h = hh * 8 + h8
                            nc.tensor.matmul(
                                ps_o[:8, :],
                                attnT[:, h * 8:(h + 1) * 8],
                                wo_t[:, h8 * 512:(h8 + 1) * 512],
                                start=(h == 0), stop=(h == HP - 1),
                            )
                    nc.vector.tensor_copy(obuf[:, i * 512:(i + 1) * 512], ps_o[:8, :])
                ob = dram.tile([B, nch * 512], F32, tag=f"ob{n}", name=f"ob{n}")
                nc.sync.dma_start(out=ob[:], in_=obuf[:])
                ors = dram.tile([1, nch * 512], F32, tag=f"ors{n}", name=f"ors{n}")
                nc.gpsimd.collective_compute(
                    "ReduceScatter",
                    mybir.AluOpType.add,
                    replica_groups=rg,
                    ins=[ob.opt()],
                    outs=[ors.opt()],
                )
                nc.gpsimd.dma_start(
                    out=o[:, ci0 * 512:(ci0 + nch) * 512], in_=ors[:]
                )
                ci0 += nch

    nc.compile()
    return nc


_NC_CACHE = None


def _get_nc():
    global _NC_CACHE
    if _NC_CACHE is None:
        _NC_CACHE = build_nc()
    return _NC_CACHE


def make_in_maps(x, k_cache, v_cache, Wq_down, Wq_up, Wkv_down, Wv_up, Wo):
    x2 = np.asarray(x, dtype=np.float32).reshape(B, HID)
    k_cache = np.asarray(k_cache, dtype=np.float32)
    v_cache = np.asarray(v_cache, dtype=np.float32)
    wd_full = np.concatenate(
        [np.asarray(Wq_down, dtype=np.float32), np.asarray(Wkv_down, dtype=np.float32)],
        axis=1,
    )  # [7168, 2048]
    Wq_up = np.asarray(Wq_up, dtype=np.float32)
    Wv_up = np.asarray(Wv_up, dtype=np.float32)
    Wo = np.asarray(Wo, dtype=np.float32)

    in_maps = []
    for c in range(NC_):
        hs = slice(c * HP, (c + 1) * HP)
        cols = slice(c * NH, (c + 1) * NH)
        rows = slice(c * HROWS, (c + 1) * HROWS)
        xt_c = np.ascontiguousarray(
            x2[:, rows].T.reshape(7, 128, B).transpose(1, 0, 2).reshape(128, 7 * B)
        ).astype(NPBF16)
        wd_c = np.ascontiguousarray(
            wd_full[rows].reshape(7, 128, 2048).transpose(1, 0, 2).reshape(128, 7 * 2048)
        ).astype(NPBF16)
        wq_c = np.ascontiguousarray(
            Wq_up[:, cols].reshape(12, 128, 2048).transpose(1, 0, 2).reshape(128, 12 * 2048)
        ).astype(NPBF16)
        wv_c = np.ascontiguousarray(
            Wv_up[:, cols].reshape(4, 128, 2048).transpose(1, 0, 2).reshape(128, 4 * 2048)
        ).astype(NPBF16)
        # kt tile t = head t: [d, (b, keys)]
        kt_c = np.ascontiguousarray(
            k_cache[:, hs].transpose(1, 3, 0, 2).reshape(16, 128, 4096)
        ).astype(NPBF16)
        # v tile t = head t: [l-in-chunk, (chunk, b, d)]
        v_c = np.ascontiguousarray(
            v_cache[:, hs]                     # (8 b, 16 h, 512 l, 128 d)
            .reshape(B, HP, 4, 128, 128)       # [b, h, cc, l, d]
            .transpose(1, 3, 2, 0, 4)          # [h, l, cc, b, d]
            .reshape(16, 128, 4096)
        ).astype(NPBF16)
        # wo chunk-major halves: [(n,i,hh) 28, d 128, (h8, 512)]
        wo_c = np.ascontiguousarray(
            Wo[cols].reshape(2, 8, 128, 14, 512)   # [hh, h8, d, k, j]
            .transpose(3, 0, 2, 1, 4)              # [k, hh, d, h8, j]
            .reshape(28, 128, 4096)
        ).astype(NPBF16)
        in_maps.append(
            {
                "xt": xt_c,
                "wd": wd_c,
                "wq": wq_c,
                "wv": wv_c,
                "kt": kt_c,
                "v": v_c,
                "wo": wo_c,
            }
        )
    return in_maps


def kernel(x, k_cache, v_cache, Wq_down, Wq_up, Wkv_down, Wk_up, Wv_up, Wo, **_):
    in_maps = make_in_maps(
        x, k_cache, v_cache, Wq_down, Wq_up, Wkv_down, Wv_up, Wo
    )
    nc = _get_nc()
    res = bass_utils.run_bass_kernel_spmd(nc, in_maps, core_ids=list(range(NC_)))
    out = np.stack([res.results[b]["o"] for b in range(B)], axis=0)  # (8, 1, 7168)
    return np.ascontiguousarray(out, dtype=np.float32)
